# revision 7
# baseline (speedup 1.0000x reference)
"""Per-pixel adaptive 5x5 conv (KPN apply) on 8 Trainium2 NeuronCores.

out[b,c,h,w] = sum_{i,j} core[b,0,i*5+j,c,h,w] * frames[b,0,c,h+i-2,w+j-2]
(zero-padded borders), output [4,3,512,512] f32.

Sharding: pure data parallel, core k -> (b = k//2, H-half = k%2).
Each NeuronCore gets a zero-padded frame shard [3, 260, 516] (halo rows +
W padding done on host), a core shard [25, 3, 256, 512], and produces
[3, 256, 512].

Raw-bass implementation (the walrus build in this env only allows one
semaphore wait per compute/DMA instruction, so Tile's auto-sync can't be
used): explicit double-buffered pipeline, standalone waits (one condition
per instruction), all loads/stores on the SP HWDGE FIFO so ordering among
DMAs is implicit.

Per 128-row block: one DMA brings all 25 tap planes [128, 25, 512], one
DMA brings a 5-row overlapping window of the padded frame [128, 5, 516];
DVE does 25 products (tap row-shift i = window slice, col-shift j =
free-dim element offset) and a serial accumulate chain.
"""

import numpy as np

import concourse.bass as bass
import concourse.mybir as mybir
from concourse.ap import AP
from concourse.bass_utils import run_bass_kernel_spmd

B, N, C, H, W = 4, 1, 3, 512, 512
K = 5
PAD = K // 2
NCORES = 8
HH = H // (NCORES // B)  # 256 rows per core
P = 128
NBLK_TOT = C * (HH // P)  # 6 blocks of 128 rows per core
WPAD = W + 2 * PAD        # 516

_CACHE = {}


def _build():
    nc = bass.Bass()
    f32 = mybir.dt.float32

    fr = nc.declare_dram_parameter("fr", [C, HH + 2 * PAD, WPAD], f32, isOutput=False)
    co = nc.declare_dram_parameter("co", [K * K, C, HH, W], f32, isOutput=False)
    out = nc.declare_dram_parameter("out", [C, HH, W], f32, isOutput=True)

    def co_view(n):
        c, blk = n // (HH // P), n % (HH // P)
        return co[:, c, blk * P:blk * P + P, :].transpose([1, 0, 2])

    def fr_win(n):
        c, blk = n // (HH // P), n % (HH // P)
        fb = fr[c, blk * P:blk * P + P, :]
        return AP(fb.tensor, fb.offset, [(WPAD, P), (WPAD, K), (1, WPAD)])

    def out_view(n):
        c, blk = n // (HH // P), n % (HH // P)
        return out[c, blk * P:blk * P + P, :]

    with (
        nc.sbuf_tensor("ct0", [P, K * K, W], f32) as ct0,
        nc.sbuf_tensor("ct1", [P, K * K, W], f32) as ct1,
        nc.sbuf_tensor("ft0", [P, K, WPAD], f32) as ft0,
        nc.sbuf_tensor("ft1", [P, K, WPAD], f32) as ft1,
        nc.sbuf_tensor("ac0", [P, W], f32) as ac0,
        nc.sbuf_tensor("ac1", [P, W], f32) as ac1,
        nc.sbuf_tensor("tmp", [P, W], f32) as tmp,
        nc.semaphore("dsem") as dsem,   # load completions (+16 per DMA)
        nc.semaphore("osem") as osem,   # store completions (+16 per DMA)
        nc.semaphore("vsem") as vsem,   # DVE per-block completion (+1)
        nc.Block() as block,
    ):
        cts, fts, acs = [ct0, ct1], [ft0, ft1], [ac0, ac1]

        @block.sync
        def _(sync: bass.BassEngine):
            for n in range(NBLK_TOT):
                if n >= 2:
                    # DVE done with block n-2 => its ct/ft buffers reusable,
                    # and acc[n-2] ready to store.
                    sync.wait_ge(vsem, n - 1)
                    sync.dma_start(
                        out=out_view(n - 2), in_=acs[n % 2][:]
                    ).then_inc(osem, 16)
                sync.dma_start(out=cts[n % 2][:], in_=co_view(n)).then_inc(dsem, 16)
                sync.dma_start(out=fts[n % 2][:], in_=fr_win(n)).then_inc(dsem, 16)
            sync.wait_ge(vsem, NBLK_TOT - 1)
            sync.dma_start(
                out=out_view(NBLK_TOT - 2), in_=acs[NBLK_TOT % 2][:]
            ).then_inc(osem, 16)
            sync.wait_ge(vsem, NBLK_TOT)
            sync.dma_start(
                out=out_view(NBLK_TOT - 1), in_=acs[(NBLK_TOT + 1) % 2][:]
            ).then_inc(osem, 16)
            sync.wait_ge(osem, 16 * NBLK_TOT)

        @block.vector
        def _(vector: bass.BassEngine):
            for n in range(NBLK_TOT):
                ct, ft, acc = cts[n % 2], fts[n % 2], acs[n % 2]
                vector.wait_ge(dsem, 32 * (n + 1))
                if n >= 2:
                    # store of block n-2 (same acc buffer) must be done
                    vector.wait_ge(osem, 16 * (n - 1))
                for t in range(K * K):
                    i, j = t // K, t % K
                    csl = ct[:, t, :]
                    fsl = ft[:, i, j:j + W]
                    if t == 0:
                        vector.tensor_mul(out=acc[:], in0=csl, in1=fsl)
                    else:
                        vector.tensor_mul(out=tmp[:], in0=csl, in1=fsl)
                        ins = vector.tensor_add(out=acc[:], in0=acc[:], in1=tmp[:])
                        if t == K * K - 1:
                            ins.then_inc(vsem, 1)
    return nc


def get_nc():
    if "nc" not in _CACHE:
        _CACHE["nc"] = _build()
    return _CACHE["nc"]


def shard_inputs(frames, core):
    frames = np.asarray(frames, dtype=np.float32)
    core = np.asarray(core, dtype=np.float32)
    in_maps = []
    for k in range(NCORES):
        b, half = k // 2, k % 2
        h0 = half * HH
        frp = np.zeros((C, HH + 2 * PAD, WPAD), np.float32)
        lo, hi = h0 - PAD, h0 + HH + PAD
        clo, chi = max(lo, 0), min(hi, H)
        frp[:, clo - lo:clo - lo + chi - clo, PAD:PAD + W] = frames[b, 0, :, clo:chi, :]
        in_maps.append({
            "fr": frp,
            "co": np.ascontiguousarray(core[b, 0, :, :, h0:h0 + HH, :]),
        })
    return in_maps


def run(in_maps, **kwargs):
    return run_bass_kernel_spmd(get_nc(), in_maps, list(range(NCORES)), **kwargs)


def kernel(frames, core):
    in_maps = shard_inputs(frames, core)
    res = run(in_maps).results
    outp = np.empty((B, C, H, W), np.float32)
    for k in range(NCORES):
        b, half = k // 2, k % 2
        outp[b, :, half * HH:(half + 1) * HH, :] = res[k]["out"]
    return outp
